# revision 34
# baseline (speedup 1.0000x reference)
"""Trainium2 Bass kernel for nn_AttnBlock_12704513262242.

Math (per sample b, W=2048 positions with scalar q/k values):
  h   = layernorm(x) * gamma + beta
  q,k,v = h @ W* + b*
  attn  = softmax(-|q_j - k_i|, over i)
  h2[j] = sum_i attn[j,i] * v[i]
  out   = x + h2 @ Wp + bp

Sharding: feature-parallel QKV (each core owns a 256-col slice of the fused
[W,768] qkv weight), AllToAll to sample-major, data-parallel attention
(4 samples/core), AllGather of h2 (transposed layout), feature-sliced
output projection.  Host concatenates the 8 [32,256] slices.

Attention (mode "fast"): the softmin kernel exp(-|q-k|) factorizes as
  e^{-q}e^{k} (k<=q) + e^{q}e^{-k} (k>q).
Build per-sample prefix tables at G grid points g with ONE 0/1 mask matmul
over 4 columns [e^k v, e^k, e^-k v, e^-k]; suffix sums come from
total - prefix (total = prefix at the last grid point).  Fold e^{-+g} into
the tables, then for each query j with nearest grid point g_j and offset
d = q_j - g_j:
  num(q) = e^{-d}U + e^{d}V ~= (U+V) + d(V-U) = T0[g_j] + d*T1[g_j]
(first order in |d| <= DELTA/2), evaluated with a one-hot matmul whose
output lands TRANSPOSED ([128,16] per sample) so the division, h2 exchange
and output projection all stay in matmul-friendly layouts.  gamma/beta are
folded into the weights host-side.  Weights travel as fp8e4m3 (adds ~2e-3
rel err, tolerance is 2e-2).
"""

import os
import sys

import numpy as np

for _p in ("/opt/trn_rl_repo", "/root/.axon_site/_ro/trn_rl_repo"):
    if os.path.isdir(_p) and _p not in sys.path:
        sys.path.insert(0, _p)

import concourse.bass as bass
import concourse.tile as tile
from concourse import bacc, mybir
from concourse.bass_utils import run_bass_kernel_spmd

F32 = mybir.dt.float32
F16 = mybir.dt.float16
F8 = mybir.dt.float8e4
ALU = mybir.AluOpType
ACTF = mybir.ActivationFunctionType

B = 32
W = 2048
NCORES = 8
PCH = W // 128     # 16 feature chunks
FSL = W // NCORES  # 256
QKVW = 3 * FSL     # 768
SPC = B // NCORES  # 4 samples per core

G = 128
LO, HI = -8.0, 8.0
DELTA = (HI - LO) / (G - 1)
HALF = DELTA / 2.0
EPS = 1e-6

MODE = os.environ.get("ATTN_MODE", "fast")
MASK1 = os.environ.get("MASK1", "1") == "1"   # single stride-0 mask op
GROUPS = [list(range(NCORES))]


def _ap(handle, offset, ap):
    return bass.AP(tensor=handle, offset=offset, ap=ap)


def build(mode=None, reps=1, skip_gb=True):
    mode = mode or MODE
    nc = bacc.Bacc("TRN2", target_bir_lowering=False, debug=False,
                   num_devices=NCORES)

    x_t = nc.dram_tensor("x", [B, W], F16, kind="ExternalInput")
    wqkv_t = nc.dram_tensor("wqkv", [W, QKVW], F8, kind="ExternalInput")
    bqkv_t = nc.dram_tensor("bqkv", [QKVW], F16, kind="ExternalInput")
    wp_t = nc.dram_tensor("wp", [W, FSL], F8, kind="ExternalInput")
    xbp_t = nc.dram_tensor("xbp", [B, FSL], F32, kind="ExternalInput")
    out_t = nc.dram_tensor("out", [B, FSL], F32, kind="ExternalOutput")

    qkv_loc = nc.dram_tensor("qkv_loc", [B, QKVW], F16)
    qkv_a2a = nc.dram_tensor("qkv_a2a", [B, QKVW], F16)
    h2_loc = nc.dram_tensor("h2_loc", [128, SPC * PCH], F16)
    h2_gat = nc.dram_tensor("h2_gat", [NCORES * 128, SPC * PCH], F16,
                            addr_space="Shared")

    gridv = np.linspace(LO, HI, G, dtype=np.float64)
    c_eye8 = nc.inline_tensor(np.eye(8, dtype=np.float16), "c_eye8")
    c_gcoln = nc.inline_tensor((-gridv).astype(np.float32).reshape(G, 1),
                               "c_gcoln")
    c_gf16 = nc.inline_tensor(gridv.astype(np.float16).reshape(G, 1),
                              "c_gf16")
    c_egm = nc.inline_tensor(np.exp(-gridv).astype(np.float32).reshape(G, 1),
                             "c_egm")
    c_egp = nc.inline_tensor(np.exp(gridv).astype(np.float32).reshape(G, 1),
                             "c_egp")
    e127 = np.zeros((128, G), np.float32)
    e127[G - 1, :] = 1.0
    c_e127 = nc.inline_tensor(e127, "c_e127")
    c_gbig = nc.inline_tensor(
        np.repeat(gridv.astype(np.float16), PCH).reshape(1, G * PCH),
        "c_gbig")
    c_grow16 = nc.inline_tensor(gridv.astype(np.float16).reshape(1, G),
                                "c_grow16")

    aps = dict(
        x=x_t.ap(), wqkv=wqkv_t.ap(), bqkv=bqkv_t, wp=wp_t.ap(),
        xbp=xbp_t.ap(), out=out_t.ap(),
        qkv_loc=qkv_loc, qkv_a2a=qkv_a2a, h2_loc=h2_loc, h2_gat=h2_gat,
        eye8=c_eye8.ap(), gcoln=c_gcoln.ap(),
        gf16=c_gf16.ap(), egm=c_egm.ap(), egp=c_egp.ap(),
        e127=c_e127.ap(), gbig=c_gbig, grow16=c_grow16.ap(),
    )

    with tile.TileContext(nc) as tc:
        with tc.tile_pool(name="const", bufs=1) as cpool:
            cst = {}
            cst["eye8"] = cpool.tile([8, 8], F16, name="c_eye8s")
            nc.sync.dma_start(cst["eye8"][:], aps["eye8"])
            cst["gcoln"] = cpool.tile([G, 1], F32, name="c_gcolns")
            nc.sync.dma_start(cst["gcoln"][:], aps["gcoln"])
            cst["gf16"] = cpool.tile([G, 1], F16, name="c_gf16s")
            nc.sync.dma_start(cst["gf16"][:], aps["gf16"])
            cst["egm"] = cpool.tile([G, 1], F32, name="c_egms")
            nc.sync.dma_start(cst["egm"][:], aps["egm"])
            cst["egp"] = cpool.tile([G, 1], F32, name="c_egps")
            nc.sync.dma_start(cst["egp"][:], aps["egp"])
            cst["e127"] = cpool.tile([128, G], F32, name="c_e127s")
            nc.sync.dma_start(cst["e127"][:], aps["e127"])
            if MASK1:
                gbig_flat = cpool.tile([128, G * PCH], F16)
                nc.gpsimd.dma_start(
                    gbig_flat[:],
                    aps["gbig"].ap().partition_broadcast(128))
                a = gbig_flat[:]
                cst["gbigT"] = bass.AP(
                    tensor=a.tensor, offset=a.offset,
                    ap=[a.ap[0], [PCH, G], [1, PCH]])
            else:
                cst["gbc"] = cpool.tile([128, G], F16, name="c_gbcs")
                nc.gpsimd.dma_start(
                    cst["gbc"][:], aps["grow16"].partition_broadcast(128))
            for _rep in range(reps):
                _build_rep(tc, aps, cst)

    nc.compile()
    return nc


def _build_rep(tc, aps, cst):
    nc = tc.nc

    with tc.tile_pool(name="main", bufs=2) as mp, \
         tc.tile_pool(name="wpool", bufs=2) as wp_pool, \
         tc.tile_pool(name="qbp", bufs=2) as qb_pool, \
         tc.tile_pool(name="att", bufs=2) as ap_, \
         tc.tile_pool(name="ps_tr", bufs=2, space="PSUM") as ps_tr, \
         tc.tile_pool(name="ps_q", bufs=1, space="PSUM") as ps_q, \
         tc.tile_pool(name="ps_s", bufs=1, space="PSUM") as ps_s, \
         tc.tile_pool(name="ps_pj", bufs=2, space="PSUM") as ps_pj:

        # ---------------- layernorm ----------------
        sbx = mp.tile([B, W], F16, tag="sbx")
        nc.sync.dma_start(sbx[:], aps["x"])
        xg = sbx[:].rearrange("b (s f) -> b s f", s=4)
        stats = mp.tile([B, 4, 6], F32, tag="stats")
        for sg in range(4):
            nc.vector.bn_stats(stats[:, sg, :], xg[:, sg, :])
        mv = mp.tile([B, 2], F32, tag="mv")
        nc.vector.bn_aggr(mv[:], stats[:])
        eps_t = mp.tile([B, 1], F32, tag="eps")
        nc.vector.memset(eps_t[:], EPS)
        lnv = mp.tile([B, 1], F32, tag="lnv")
        nc.scalar.activation(lnv[:], mv[:, 1:2], ACTF.Ln, bias=eps_t[:])
        rstd = mp.tile([B, 1], F32, tag="rstd")
        nc.scalar.activation(rstd[:], lnv[:], ACTF.Exp, scale=-0.5)
        h = mp.tile([B, W], F16, tag="h")
        nc.vector.tensor_scalar(h[:], sbx[:], mv[:, 0:1], rstd[:],
                                op0=ALU.subtract, op1=ALU.mult)

        # -------- h transpose via DMA xbar: hT[p, ci, b] = h[b, ci*128+p] --
        hT = mp.tile([128, PCH, B], F16, tag="hT")
        nc.sync.dma_start_transpose(hT[:], h[:])

        # ---------------- qkv matmul ----------------
        wq_t = wp_pool.tile([128, PCH, QKVW], F8, tag="wq")
        nc.sync.dma_start(
            wq_t[:], _ap(aps["wqkv"].tensor, 0,
                         [[QKVW, 128], [128 * QKVW, PCH], [1, QKVW]]))
        pq = ps_q.tile([B, QKVW], F32, tag="pq")
        for ci in range(PCH):
            nc.tensor.matmul(pq[:, 0:512], hT[:, ci, :], wq_t[:, ci, 0:512],
                             start=(ci == 0), stop=(ci == PCH - 1))
            nc.tensor.matmul(pq[:, 512:QKVW], hT[:, ci, :],
                             wq_t[:, ci, 512:QKVW],
                             start=(ci == 0), stop=(ci == PCH - 1))
        bqb = mp.tile([B, QKVW], F16, tag="bqb")
        nc.gpsimd.dma_start(bqb[:], aps["bqkv"].ap().partition_broadcast(B))
        sbq = mp.tile([B, QKVW], F16, tag="sbq")
        nc.vector.tensor_tensor(sbq[:], pq[:], bqb[:], op=ALU.add)
        nc.sync.dma_start(aps["qkv_loc"].ap(), sbq[:])

        # ---------------- AllToAll ----------------
        nc.gpsimd.collective_compute(
            "AllToAll", ALU.bypass, replica_groups=GROUPS,
            ins=[aps["qkv_loc"].ap()], outs=[aps["qkv_a2a"].ap()])

        a2a = aps["qkv_a2a"]

        # q rows broadcast: qb [128, SPC, W] f16 (one DMA per sample)
        qb = qb_pool.tile([128, SPC, W], F16, tag="qb")
        qb_eng = [nc.scalar, nc.vector, nc.scalar, nc.vector]
        for s in range(SPC):
            qb_eng[s].dma_start(
                qb[:, s, :],
                _ap(a2a, s * QKVW, [[0, 128], [4 * QKVW, 8], [1, FSL]]))
        # q/k/v rows natural: rows [8, SPC, 3, 256]
        rows = ap_.tile([8, SPC, 3, FSL], F16, tag="rows")
        for s in range(SPC):
            nc.gpsimd.dma_start(
                rows[:, s, :, :],
                _ap(a2a, s * QKVW, [[4 * QKVW, 8], [FSL, 3], [1, FSL]]))

        h2a = mp.tile([128, SPC, PCH], F16, tag="h2a")

        for s in range(SPC):
            # --- transpose q/k/v to [128, 16] ---
            tqkv = ap_.tile([128, 3, PCH], F16, tag="tqkv")
            for w in range(3):
                for hh in range(2):
                    ptq = ps_tr.tile([128, 8], F16, tag="ptq")
                    nc.tensor.transpose(
                        ptq[:], rows[:, s, w, hh * 128:(hh + 1) * 128],
                        cst["eye8"][:])
                    base = tqkv[:, w, :]
                    dst = bass.AP(tensor=base.tensor,
                                  offset=base.offset + hh,
                                  ap=[base.ap[0], [2, 8]])
                    if (w + hh) % 2 == 0:
                        nc.scalar.copy(dst, ptq[:])
                    else:
                        nc.vector.tensor_copy(dst, ptq[:])
            qT = tqkv[:, 0, :]
            kc = ap_.tile([128, PCH], F16, tag="kc")
            nc.vector.tensor_scalar(kc[:], tqkv[:, 1, :], LO, HI,
                                    op0=ALU.max, op1=ALU.min)

            # --- u columns [e^k v, e^k, e^-k v, e^-k] ---
            u = ap_.tile([128, PCH, 4], F16, tag="u")
            nc.scalar.activation(u[:, :, 1], kc[:], ACTF.Exp)
            nc.scalar.activation(u[:, :, 3], kc[:], ACTF.Exp, scale=-1.0)
            nc.vector.tensor_tensor(u[:, :, 0], u[:, :, 1], tqkv[:, 2, :],
                                    op=ALU.mult)
            nc.vector.tensor_tensor(u[:, :, 2], u[:, :, 3], tqkv[:, 2, :],
                                    op=ALU.mult)

            # --- mask + prefix tables S[g, r] ---
            if MASK1:
                mk = ap_.tile([128, G, PCH], F16, tag="mk")
                ka = kc[:]
                kb0 = bass.AP(tensor=ka.tensor, offset=ka.offset,
                              ap=[ka.ap[0], [0, G], [1, PCH]])
                mk_eng = nc.vector if s % 2 == 0 else nc.gpsimd
                mk_eng.tensor_tensor(mk[:], kb0, cst["gbigT"],
                                     ALU.is_le)

                def mk_lhsT(m):
                    a = mk[:]
                    return bass.AP(tensor=a.tensor, offset=a.offset + m,
                                   ap=[a.ap[0], [PCH, G]])
            else:
                mk = ap_.tile([128, PCH, G], F16, tag="mk")
                for m in range(PCH):
                    nc.vector.tensor_scalar(mk[:, m, :], cst["gbc"][:],
                                            kc[:, m:m + 1], None,
                                            op0=ALU.is_ge)

                def mk_lhsT(m):
                    return mk[:, m, :]

            SS = ps_s.tile([G, 8], F32, tag="SS")
            S = SS[:, 0:4]
            TBp = SS[:, 4:6]
            for m in range(PCH):
                nc.tensor.matmul(S, mk_lhsT(m), u[:, m, :],
                                 start=(m == 0), stop=(m == PCH - 1))
            Ssb = ap_.tile([G, 4], F32, tag="Ssb")
            nc.scalar.copy(Ssb[:], S)

            # broadcast totals (= prefix at the top grid row) to all rows
            nc.tensor.matmul(TBp, cst["e127"][:], Ssb[:, 2:4],
                             start=True, stop=True)
            V2 = ap_.tile([G, 2], F32, tag="V2")
            nc.vector.tensor_tensor(V2[:], TBp, Ssb[:, 2:4],
                                    op=ALU.subtract)
            U = ap_.tile([G, 2], F32, tag="U")
            nc.gpsimd.tensor_scalar(U[:], Ssb[:, 0:2], cst["egm"][:], None,
                                    op0=ALU.mult)
            V = ap_.tile([G, 2], F32, tag="V")
            nc.gpsimd.tensor_scalar(V[:], V2[:], cst["egp"][:], None,
                                    op0=ALU.mult)
            tabs = ap_.tile([G, 5], F16, tag="tabs")
            nc.vector.tensor_tensor(tabs[:, 0:2], U[:], V[:], op=ALU.add)
            nc.vector.tensor_tensor(tabs[:, 2:4], V[:], U[:],
                                    op=ALU.subtract)
            nc.gpsimd.tensor_copy(tabs[:, 4:5], cst["gf16"][:])

            # --- one-hot E ---
            t1 = qb_pool.tile([128, W], F16, tag="t1")
            nc.scalar.activation(t1[:], qb[:, s, :], ACTF.Abs,
                                 bias=cst["gcoln"][:])
            E = qb_pool.tile([128, W], F16, tag="E")
            nc.vector.tensor_scalar(E[:], t1[:], HALF, None, op0=ALU.is_le)

            # --- gather (transposed): PJ [128, 16, 5] ---
            PJ = ps_pj.tile([128, PCH, 5], F32, tag="PJ")
            for mj in range(PCH):
                nc.tensor.matmul(PJ[:, mj, :],
                                 E[:, mj * 128:(mj + 1) * 128], tabs[:],
                                 start=True, stop=True)

            # --- combine: h2 = (T0 + d T1) / (T2 + d T3) ---
            dn = ap_.tile([128, PCH], F32, tag="dn")
            nc.vector.tensor_tensor(dn[:], qT, PJ[:, :, 4], op=ALU.subtract)
            cn = ap_.tile([128, PCH], F32, tag="cn")
            nc.vector.tensor_tensor(cn[:], dn[:], PJ[:, :, 2], op=ALU.mult)
            num = ap_.tile([128, PCH], F32, tag="num")
            nc.vector.tensor_tensor(num[:], cn[:], PJ[:, :, 0], op=ALU.add)
            cd = ap_.tile([128, PCH], F32, tag="cd")
            nc.vector.tensor_tensor(cd[:], dn[:], PJ[:, :, 3], op=ALU.mult)
            den = ap_.tile([128, PCH], F32, tag="den")
            nc.vector.tensor_tensor(den[:], cd[:], PJ[:, :, 1], op=ALU.add)
            rden = ap_.tile([128, PCH], F32, tag="rden")
            nc.vector.reciprocal(rden[:], den[:])
            nc.vector.tensor_tensor(h2a[:, s, :], num[:], rden[:],
                                    op=ALU.mult)

        nc.gpsimd.dma_start(aps["h2_loc"].ap(), h2a[:])

        # ---------------- AllGather (transposed h2) ----------------
        nc.gpsimd.collective_compute(
            "AllGather", ALU.bypass, replica_groups=GROUPS,
            ins=[aps["h2_loc"].ap()], outs=[aps["h2_gat"].ap()])

        # h2g [128, 8, 64]: partition p, core c, (s*16+m)
        SW = SPC * PCH
        h2g = mp.tile([128, NCORES, SW], F16, tag="h2g")
        nc.sync.dma_start(
            h2g[:], _ap(aps["h2_gat"], 0,
                        [[SW, 128], [128 * SW, NCORES], [1, SW]]))

        # ---------------- output projection ----------------
        wpb = wp_pool.tile([128, PCH, FSL], F8, tag="wpb")
        nc.sync.dma_start(
            wpb[:], _ap(aps["wp"].tensor, 0,
                        [[FSL, 128], [128 * FSL, PCH], [1, FSL]]))
        xbp_sb = mp.tile([B, FSL], F32, tag="xbp")
        nc.gpsimd.dma_start(xbp_sb[:], aps["xbp"])
        pout = ps_q.tile([B, FSL], F32, tag="pout")
        for ci in range(PCH):
            a = h2g[:]
            lhsT = bass.AP(tensor=a.tensor, offset=a.offset + ci,
                           ap=[a.ap[0], [SW, NCORES], [PCH, SPC]])
            nc.tensor.matmul(pout[:], lhsT, wpb[:, ci, :],
                             start=(ci == 0), stop=(ci == PCH - 1))
        outsb = mp.tile([B, FSL], F32, tag="outsb")
        nc.vector.tensor_tensor(outsb[:], pout[:], xbp_sb[:], op=ALU.add)
        nc.sync.dma_start(aps["out"], outsb[:])


_BUILT = {}


def _get_nc(mode, skip_gb=True):
    key = (mode, skip_gb)
    if key not in _BUILT:
        _BUILT[key] = build(mode, skip_gb=skip_gb)
    return _BUILT[key]


def make_in_maps(inputs):
    x = np.ascontiguousarray(np.asarray(inputs["x"], np.float32))
    gamma = np.asarray(inputs["gamma"], np.float32)
    beta = np.asarray(inputs["beta"], np.float32)
    Wq = np.asarray(inputs["Wq"], np.float32)
    Wk = np.asarray(inputs["Wk"], np.float32)
    Wv = np.asarray(inputs["Wv"], np.float32)
    Wp = np.asarray(inputs["Wp"], np.float32)
    bq = np.asarray(inputs["bq"], np.float32)
    bk = np.asarray(inputs["bk"], np.float32)
    bv = np.asarray(inputs["bv"], np.float32)
    bp = np.asarray(inputs["bp"], np.float32)

    # fold gamma/beta into the qkv weights: (h*gamma + beta) @ W + b
    #   = h @ (gamma[:,None]*W) + (beta @ W + b)
    g = gamma[:, None]
    Wqf, Wkf, Wvf = g * Wq, g * Wk, g * Wv
    bqf = beta @ Wq + bq
    bkf = beta @ Wk + bk
    bvf = beta @ Wv + bv

    f8 = mybir.dt.np(F8)
    in_maps = []
    for c in range(NCORES):
        cs = slice(c * FSL, (c + 1) * FSL)
        wqkv = np.concatenate([Wqf[:, cs], Wkf[:, cs], Wvf[:, cs]], axis=1)
        bqkv = np.concatenate([bqf[cs], bkf[cs], bvf[cs]])
        in_maps.append({
            "x": x.astype(np.float16),
            "wqkv": np.ascontiguousarray(wqkv).astype(f8),
            "bqkv": np.ascontiguousarray(bqkv).astype(np.float16),
            "wp": np.ascontiguousarray(Wp[:, cs]).astype(f8),
            "xbp": np.ascontiguousarray(x[:, cs] + bp[None, cs]),
        })
    return in_maps


def kernel(**inputs):
    nc = _get_nc(MODE)
    in_maps = make_in_maps(inputs)
    res = run_bass_kernel_spmd(nc, in_maps, core_ids=list(range(NCORES)))
    out = np.concatenate([res.results[c]["out"] for c in range(NCORES)],
                         axis=1)
    return np.ascontiguousarray(out.astype(np.float32))


# revision 38
# speedup vs baseline: 4.3056x; 4.3056x over previous
"""Trainium2 Bass kernel for nn_AttnBlock_12704513262242.

Math (per sample b, W=2048 positions with scalar q/k values):
  h   = layernorm(x) * gamma + beta
  q,k,v = h @ W* + b*
  attn  = softmax(-|q_j - k_i|, over i)
  h2[j] = sum_i attn[j,i] * v[i]
  out   = x + h2 @ Wp + bp

Sharding: feature-parallel QKV (each core owns a 256-col slice of the fused
[W,768] qkv weight), AllToAll to sample-major, data-parallel attention
(4 samples/core), AllGather of h2 (transposed layout), feature-sliced
output projection.  Host concatenates the 8 [32,256] slices.

Attention (mode "fast"): the softmin kernel exp(-|q-k|) factorizes as
  e^{-q}e^{k} (k<=q) + e^{q}e^{-k} (k>q).
Build per-sample prefix tables at G grid points g with ONE 0/1 mask matmul
over 4 columns [e^k v, e^k, e^-k v, e^-k]; suffix sums come from
total - prefix (total = prefix at the last grid point).  Fold e^{-+g} into
the tables, then for each query j with nearest grid point g_j and offset
d = q_j - g_j:
  num(q) = e^{-d}U + e^{d}V ~= (U+V) + d(V-U) = T0[g_j] + d*T1[g_j]
(first order in |d| <= DELTA/2), evaluated with a one-hot matmul whose
output lands TRANSPOSED ([128,16] per sample) so the division, h2 exchange
and output projection all stay in matmul-friendly layouts.  gamma/beta are
folded into the weights host-side.  Weights travel as fp8e4m3 (adds ~2e-3
rel err, tolerance is 2e-2).
"""

import os
import sys

import numpy as np

for _p in ("/opt/trn_rl_repo", "/root/.axon_site/_ro/trn_rl_repo"):
    if os.path.isdir(_p) and _p not in sys.path:
        sys.path.insert(0, _p)

import concourse.bass as bass
import concourse.tile as tile
from concourse import bacc, mybir
from concourse.bass_utils import run_bass_kernel_spmd

F32 = mybir.dt.float32
F16 = mybir.dt.float16
F8 = mybir.dt.float8e4
ALU = mybir.AluOpType
ACTF = mybir.ActivationFunctionType

B = 32
W = 2048
NCORES = 8
PCH = W // 128     # 16 feature chunks
FSL = W // NCORES  # 256
QKVW = 3 * FSL     # 768
SPC = B // NCORES  # 4 samples per core

G = 128
LO, HI = -8.0, 8.0
DELTA = (HI - LO) / (G - 1)
HALF = DELTA / 2.0
EPS = 1e-6

MODE = os.environ.get("ATTN_MODE", "fast")
MASK1 = os.environ.get("MASK1", "1") == "1"   # single stride-0 mask op
GROUPS = [list(range(NCORES))]


def _ap(handle, offset, ap):
    return bass.AP(tensor=handle, offset=offset, ap=ap)


def build(mode=None, reps=1, skip_gb=True):
    mode = mode or MODE
    nc = bacc.Bacc("TRN2", target_bir_lowering=False, debug=False,
                   num_devices=NCORES)

    x_t = nc.dram_tensor("x", [B, W], F16, kind="ExternalInput")
    wqkv_t = nc.dram_tensor("wqkv", [W, QKVW], F8, kind="ExternalInput")
    bqkv_t = nc.dram_tensor("bqkv", [QKVW], F16, kind="ExternalInput")
    wp_t = nc.dram_tensor("wp", [W, FSL], F8, kind="ExternalInput")
    xbp_t = nc.dram_tensor("xbp", [B, FSL], F32, kind="ExternalInput")
    out_t = nc.dram_tensor("out", [B, FSL], F32, kind="ExternalOutput")

    qkv_loc = nc.dram_tensor("qkv_loc", [B, QKVW], F16)
    qkv_a2a = nc.dram_tensor("qkv_a2a", [B, QKVW], F16)
    h2_loc = nc.dram_tensor("h2_loc", [128, SPC * PCH], F16)
    h2_gat = nc.dram_tensor("h2_gat", [NCORES * 128, SPC * PCH], F16,
                            addr_space="Shared")

    gridv = np.linspace(LO, HI, G, dtype=np.float64)
    c_eye8 = nc.inline_tensor(np.eye(8, dtype=np.float16), "c_eye8")
    c_gcoln = nc.inline_tensor((-gridv).astype(np.float32).reshape(G, 1),
                               "c_gcoln")
    c_gf16 = nc.inline_tensor(gridv.astype(np.float16).reshape(G, 1),
                              "c_gf16")
    c_egm = nc.inline_tensor(np.exp(-gridv).astype(np.float32).reshape(G, 1),
                             "c_egm")
    c_egp = nc.inline_tensor(np.exp(gridv).astype(np.float32).reshape(G, 1),
                             "c_egp")
    e127 = np.zeros((128, G), np.float32)
    e127[G - 1, :] = 1.0
    c_e127 = nc.inline_tensor(e127, "c_e127")
    c_gbig = nc.inline_tensor(
        np.repeat(gridv.astype(np.float16), PCH).reshape(1, G * PCH),
        "c_gbig")
    c_grow16 = nc.inline_tensor(gridv.astype(np.float16).reshape(1, G),
                                "c_grow16")

    aps = dict(
        x=x_t.ap(), wqkv=wqkv_t.ap(), bqkv=bqkv_t, wp=wp_t.ap(),
        xbp=xbp_t.ap(), out=out_t.ap(),
        qkv_loc=qkv_loc, qkv_a2a=qkv_a2a, h2_loc=h2_loc, h2_gat=h2_gat,
        eye8=c_eye8.ap(), gcoln=c_gcoln.ap(),
        gf16=c_gf16.ap(), egm=c_egm.ap(), egp=c_egp.ap(),
        e127=c_e127.ap(), gbig=c_gbig, grow16=c_grow16.ap(),
    )

    with tile.TileContext(nc) as tc:
        with tc.tile_pool(name="const", bufs=1) as cpool:
            cst = {}
            cst["eye8"] = cpool.tile([8, 8], F16, name="c_eye8s")
            nc.sync.dma_start(cst["eye8"][:], aps["eye8"])
            cst["gcoln"] = cpool.tile([G, 1], F32, name="c_gcolns")
            nc.sync.dma_start(cst["gcoln"][:], aps["gcoln"])
            cst["gf16"] = cpool.tile([G, 1], F16, name="c_gf16s")
            nc.sync.dma_start(cst["gf16"][:], aps["gf16"])
            cst["egm"] = cpool.tile([G, 1], F32, name="c_egms")
            nc.sync.dma_start(cst["egm"][:], aps["egm"])
            cst["egp"] = cpool.tile([G, 1], F32, name="c_egps")
            nc.sync.dma_start(cst["egp"][:], aps["egp"])
            cst["e127"] = cpool.tile([128, G], F32, name="c_e127s")
            nc.sync.dma_start(cst["e127"][:], aps["e127"])
            if MASK1:
                gbig_flat = cpool.tile([128, G * PCH], F16)
                nc.gpsimd.dma_start(
                    gbig_flat[:],
                    aps["gbig"].ap().partition_broadcast(128))
                a = gbig_flat[:]
                cst["gbigT"] = bass.AP(
                    tensor=a.tensor, offset=a.offset,
                    ap=[a.ap[0], [PCH, G], [1, PCH]])
            else:
                cst["gbc"] = cpool.tile([128, G], F16, name="c_gbcs")
                nc.gpsimd.dma_start(
                    cst["gbc"][:], aps["grow16"].partition_broadcast(128))
            for _rep in range(reps):
                _build_rep(tc, aps, cst)

    nc.compile()
    return nc


def _build_rep(tc, aps, cst):
    nc = tc.nc

    with tc.tile_pool(name="main", bufs=2) as mp, \
         tc.tile_pool(name="wpool", bufs=2) as wp_pool, \
         tc.tile_pool(name="qbp", bufs=2) as qb_pool, \
         tc.tile_pool(name="att", bufs=2) as ap_, \
         tc.tile_pool(name="ps_tr", bufs=2, space="PSUM") as ps_tr, \
         tc.tile_pool(name="ps_q", bufs=1, space="PSUM") as ps_q, \
         tc.tile_pool(name="ps_s", bufs=1, space="PSUM") as ps_s, \
         tc.tile_pool(name="ps_pj", bufs=2, space="PSUM") as ps_pj:

        # ---------------- layernorm ----------------
        sbx = mp.tile([B, W], F16, tag="sbx")
        nc.sync.dma_start(sbx[:], aps["x"])
        xg = sbx[:].rearrange("b (s f) -> b s f", s=4)
        stats = mp.tile([B, 4, 6], F32, tag="stats")
        for sg in range(4):
            nc.vector.bn_stats(stats[:, sg, :], xg[:, sg, :])
        mv = mp.tile([B, 2], F32, tag="mv")
        nc.vector.bn_aggr(mv[:], stats[:])
        eps_t = mp.tile([B, 1], F32, tag="eps")
        nc.vector.memset(eps_t[:], EPS)
        lnv = mp.tile([B, 1], F32, tag="lnv")
        nc.scalar.activation(lnv[:], mv[:, 1:2], ACTF.Ln, bias=eps_t[:])
        rstd = mp.tile([B, 1], F32, tag="rstd")
        nc.scalar.activation(rstd[:], lnv[:], ACTF.Exp, scale=-0.5)
        h = mp.tile([B, W], F16, tag="h")
        nc.vector.tensor_scalar(h[:], sbx[:], mv[:, 0:1], rstd[:],
                                op0=ALU.subtract, op1=ALU.mult)

        # -------- h transpose via DMA xbar: hT[p, ci, b] = h[b, ci*128+p] --
        hT = mp.tile([128, PCH, B], F16, tag="hT")
        nc.sync.dma_start_transpose(hT[:], h[:])

        # ---------------- qkv matmul ----------------
        wq_t = wp_pool.tile([128, PCH, QKVW], F8, tag="wq")
        nc.sync.dma_start(
            wq_t[:], _ap(aps["wqkv"].tensor, 0,
                         [[QKVW, 128], [128 * QKVW, PCH], [1, QKVW]]))
        pq = ps_q.tile([B, QKVW], F32, tag="pq")
        for ci in range(PCH):
            nc.tensor.matmul(pq[:, 0:512], hT[:, ci, :], wq_t[:, ci, 0:512],
                             start=(ci == 0), stop=(ci == PCH - 1))
            nc.tensor.matmul(pq[:, 512:QKVW], hT[:, ci, :],
                             wq_t[:, ci, 512:QKVW],
                             start=(ci == 0), stop=(ci == PCH - 1))
        bqb = mp.tile([B, QKVW], F16, tag="bqb")
        nc.gpsimd.dma_start(bqb[:], aps["bqkv"].ap().partition_broadcast(B))
        sbq = mp.tile([B, QKVW], F16, tag="sbq")
        nc.vector.tensor_tensor(sbq[:], pq[:], bqb[:], op=ALU.add)
        nc.sync.dma_start(aps["qkv_loc"].ap(), sbq[:])

        # ---------------- AllToAll ----------------
        nc.gpsimd.collective_compute(
            "AllToAll", ALU.bypass, replica_groups=GROUPS,
            ins=[aps["qkv_loc"].ap()], outs=[aps["qkv_a2a"].ap()])

        a2a = aps["qkv_a2a"]

        # q rows broadcast: qb [128, SPC, W] f16 (one DMA per sample)
        qb = qb_pool.tile([128, SPC, W], F16, tag="qb")
        qb_eng = [nc.scalar, nc.sync, nc.scalar, nc.sync]
        for s in range(SPC):
            qb_eng[s].dma_start(
                qb[:, s, :],
                _ap(a2a, s * QKVW, [[0, 128], [4 * QKVW, 8], [1, FSL]]))
        # q/k/v rows natural: rows [8, SPC, 3, 256]
        rows = ap_.tile([8, SPC, 3, FSL], F16, tag="rows")
        for s in range(SPC):
            nc.gpsimd.dma_start(
                rows[:, s, :, :],
                _ap(a2a, s * QKVW, [[4 * QKVW, 8], [FSL, 3], [1, FSL]]))

        h2a = mp.tile([128, SPC, PCH], F16, tag="h2a")

        for s in range(SPC):
            # --- transpose q/k/v to [128, 16] ---
            tqkv = ap_.tile([128, 3, PCH], F16, tag="tqkv")
            for w in range(3):
                for hh in range(2):
                    ptq = ps_tr.tile([128, 8], F16, tag="ptq")
                    nc.tensor.transpose(
                        ptq[:], rows[:, s, w, hh * 128:(hh + 1) * 128],
                        cst["eye8"][:])
                    base = tqkv[:, w, :]
                    dst = bass.AP(tensor=base.tensor,
                                  offset=base.offset + hh,
                                  ap=[base.ap[0], [2, 8]])
                    if (w + hh) % 2 == 0:
                        nc.scalar.copy(dst, ptq[:])
                    else:
                        nc.vector.tensor_copy(dst, ptq[:])
            qT = tqkv[:, 0, :]
            kc = ap_.tile([128, PCH], F16, tag="kc")
            nc.vector.tensor_scalar(kc[:], tqkv[:, 1, :], LO, HI,
                                    op0=ALU.max, op1=ALU.min)

            # --- u columns [e^k v, e^k, e^-k v, e^-k] ---
            u = ap_.tile([128, PCH, 4], F16, tag="u")
            nc.scalar.activation(u[:, :, 1], kc[:], ACTF.Exp)
            nc.scalar.activation(u[:, :, 3], kc[:], ACTF.Exp, scale=-1.0)
            nc.vector.tensor_tensor(u[:, :, 0], u[:, :, 1], tqkv[:, 2, :],
                                    op=ALU.mult)
            nc.vector.tensor_tensor(u[:, :, 2], u[:, :, 3], tqkv[:, 2, :],
                                    op=ALU.mult)

            # --- mask + prefix tables S[g, r] ---
            if MASK1:
                mk = ap_.tile([128, G, PCH], F16, tag="mk")
                ka = kc[:]
                kb0 = bass.AP(tensor=ka.tensor, offset=ka.offset,
                              ap=[ka.ap[0], [0, G], [1, PCH]])
                nc.vector.tensor_tensor(mk[:], kb0, cst["gbigT"],
                                        ALU.is_le)

                def mk_lhsT(m):
                    a = mk[:]
                    return bass.AP(tensor=a.tensor, offset=a.offset + m,
                                   ap=[a.ap[0], [PCH, G]])
            else:
                mk = ap_.tile([128, PCH, G], F16, tag="mk")
                for m in range(PCH):
                    nc.vector.tensor_scalar(mk[:, m, :], cst["gbc"][:],
                                            kc[:, m:m + 1], None,
                                            op0=ALU.is_ge)

                def mk_lhsT(m):
                    return mk[:, m, :]

            SS = ps_s.tile([G, 8], F32, tag="SS")
            S = SS[:, 0:4]
            TBp = SS[:, 4:6]
            for m in range(PCH):
                nc.tensor.matmul(S, mk_lhsT(m), u[:, m, :],
                                 start=(m == 0), stop=(m == PCH - 1))
            Ssb = ap_.tile([G, 4], F32, tag="Ssb")
            nc.scalar.copy(Ssb[:], S)

            # broadcast totals (= prefix at the top grid row) to all rows
            nc.tensor.matmul(TBp, cst["e127"][:], Ssb[:, 2:4],
                             start=True, stop=True)
            V2 = ap_.tile([G, 2], F32, tag="V2")
            nc.vector.tensor_tensor(V2[:], TBp, Ssb[:, 2:4],
                                    op=ALU.subtract)
            U = ap_.tile([G, 2], F32, tag="U")
            nc.gpsimd.tensor_scalar(U[:], Ssb[:, 0:2], cst["egm"][:], None,
                                    op0=ALU.mult)
            V = ap_.tile([G, 2], F32, tag="V")
            nc.gpsimd.tensor_scalar(V[:], V2[:], cst["egp"][:], None,
                                    op0=ALU.mult)
            tabs = ap_.tile([G, 5], F16, tag="tabs")
            nc.vector.tensor_tensor(tabs[:, 0:2], U[:], V[:], op=ALU.add)
            nc.vector.tensor_tensor(tabs[:, 2:4], V[:], U[:],
                                    op=ALU.subtract)
            nc.gpsimd.tensor_copy(tabs[:, 4:5], cst["gf16"][:])

            # --- one-hot E ---
            t1 = qb_pool.tile([128, W], F16, tag="t1")
            nc.scalar.activation(t1[:], qb[:, s, :], ACTF.Abs,
                                 bias=cst["gcoln"][:])
            E = qb_pool.tile([128, W], F16, tag="E")
            nc.vector.tensor_scalar(E[:], t1[:], HALF, None, op0=ALU.is_le)

            # --- gather (transposed): PJ [128, 16, 5] ---
            PJ = ps_pj.tile([128, PCH, 5], F32, tag="PJ")
            for mj in range(PCH):
                nc.tensor.matmul(PJ[:, mj, :],
                                 E[:, mj * 128:(mj + 1) * 128], tabs[:],
                                 start=True, stop=True)

            # --- combine: h2 = (T0 + d T1) / (T2 + d T3) ---
            dn = ap_.tile([128, PCH], F32, tag="dn")
            nc.vector.tensor_tensor(dn[:], qT, PJ[:, :, 4], op=ALU.subtract)
            cn = ap_.tile([128, PCH], F32, tag="cn")
            nc.vector.tensor_tensor(cn[:], dn[:], PJ[:, :, 2], op=ALU.mult)
            num = ap_.tile([128, PCH], F32, tag="num")
            nc.vector.tensor_tensor(num[:], cn[:], PJ[:, :, 0], op=ALU.add)
            cd = ap_.tile([128, PCH], F32, tag="cd")
            nc.vector.tensor_tensor(cd[:], dn[:], PJ[:, :, 3], op=ALU.mult)
            den = ap_.tile([128, PCH], F32, tag="den")
            nc.vector.tensor_tensor(den[:], cd[:], PJ[:, :, 1], op=ALU.add)
            rden = ap_.tile([128, PCH], F32, tag="rden")
            nc.vector.reciprocal(rden[:], den[:])
            nc.vector.tensor_tensor(h2a[:, s, :], num[:], rden[:],
                                    op=ALU.mult)

        nc.gpsimd.dma_start(aps["h2_loc"].ap(), h2a[:])

        # ---------------- AllGather (transposed h2) ----------------
        nc.gpsimd.collective_compute(
            "AllGather", ALU.bypass, replica_groups=GROUPS,
            ins=[aps["h2_loc"].ap()], outs=[aps["h2_gat"].ap()])

        # h2g [128, 8, 64]: partition p, core c, (s*16+m)
        SW = SPC * PCH
        h2g = mp.tile([128, NCORES, SW], F16, tag="h2g")
        nc.sync.dma_start(
            h2g[:], _ap(aps["h2_gat"], 0,
                        [[SW, 128], [128 * SW, NCORES], [1, SW]]))

        # ---------------- output projection ----------------
        wpb = wp_pool.tile([128, PCH, FSL], F8, tag="wpb")
        nc.sync.dma_start(
            wpb[:], _ap(aps["wp"].tensor, 0,
                        [[FSL, 128], [128 * FSL, PCH], [1, FSL]]))
        xbp_sb = mp.tile([B, FSL], F32, tag="xbp")
        nc.gpsimd.dma_start(xbp_sb[:], aps["xbp"])
        pout = ps_q.tile([B, FSL], F32, tag="pout")
        for ci in range(PCH):
            a = h2g[:]
            lhsT = bass.AP(tensor=a.tensor, offset=a.offset + ci,
                           ap=[a.ap[0], [SW, NCORES], [PCH, SPC]])
            nc.tensor.matmul(pout[:], lhsT, wpb[:, ci, :],
                             start=(ci == 0), stop=(ci == PCH - 1))
        outsb = mp.tile([B, FSL], F32, tag="outsb")
        nc.vector.tensor_tensor(outsb[:], pout[:], xbp_sb[:], op=ALU.add)
        nc.sync.dma_start(aps["out"], outsb[:])


_BUILT = {}


def _get_nc(mode, skip_gb=True):
    key = (mode, skip_gb)
    if key not in _BUILT:
        _BUILT[key] = build(mode, skip_gb=skip_gb)
    return _BUILT[key]


def make_in_maps(inputs):
    x = np.ascontiguousarray(np.asarray(inputs["x"], np.float32))
    gamma = np.asarray(inputs["gamma"], np.float32)
    beta = np.asarray(inputs["beta"], np.float32)
    Wq = np.asarray(inputs["Wq"], np.float32)
    Wk = np.asarray(inputs["Wk"], np.float32)
    Wv = np.asarray(inputs["Wv"], np.float32)
    Wp = np.asarray(inputs["Wp"], np.float32)
    bq = np.asarray(inputs["bq"], np.float32)
    bk = np.asarray(inputs["bk"], np.float32)
    bv = np.asarray(inputs["bv"], np.float32)
    bp = np.asarray(inputs["bp"], np.float32)

    # fold gamma/beta into the qkv weights: (h*gamma + beta) @ W + b
    #   = h @ (gamma[:,None]*W) + (beta @ W + b)
    g = gamma[:, None]
    Wqf, Wkf, Wvf = g * Wq, g * Wk, g * Wv
    bqf = beta @ Wq + bq
    bkf = beta @ Wk + bk
    bvf = beta @ Wv + bv

    f8 = mybir.dt.np(F8)
    in_maps = []
    for c in range(NCORES):
        cs = slice(c * FSL, (c + 1) * FSL)
        wqkv = np.concatenate([Wqf[:, cs], Wkf[:, cs], Wvf[:, cs]], axis=1)
        bqkv = np.concatenate([bqf[cs], bkf[cs], bvf[cs]])
        in_maps.append({
            "x": x.astype(np.float16),
            "wqkv": np.ascontiguousarray(wqkv).astype(f8),
            "bqkv": np.ascontiguousarray(bqkv).astype(np.float16),
            "wp": np.ascontiguousarray(Wp[:, cs]).astype(f8),
            "xbp": np.ascontiguousarray(x[:, cs] + bp[None, cs]),
        })
    return in_maps


def kernel(**inputs):
    nc = _get_nc(MODE)
    in_maps = make_in_maps(inputs)
    res = run_bass_kernel_spmd(nc, in_maps, core_ids=list(range(NCORES)))
    out = np.concatenate([res.results[c]["out"] for c in range(NCORES)],
                         axis=1)
    return np.ascontiguousarray(out.astype(np.float32))


# revision 40
# speedup vs baseline: 5.7064x; 1.3253x over previous
"""Trainium2 Bass kernel for nn_AttnBlock_12704513262242.

Math (per sample b, W=2048 positions with scalar q/k values):
  h   = layernorm(x) * gamma + beta
  q,k,v = h @ W* + b*
  attn  = softmax(-|q_j - k_i|, over i)
  h2[j] = sum_i attn[j,i] * v[i]
  out   = x + h2 @ Wp + bp

Sharding: feature-parallel QKV (each core owns a 256-col slice of the fused
[W,768] qkv weight), AllToAll to sample-major, data-parallel attention
(4 samples/core), AllGather of h2 (transposed layout), feature-sliced
output projection.  Host concatenates the 8 [32,256] slices.

Attention (mode "fast"): the softmin kernel exp(-|q-k|) factorizes as
  e^{-q}e^{k} (k<=q) + e^{q}e^{-k} (k>q).
Build per-sample prefix tables at G grid points g with ONE 0/1 mask matmul
over 4 columns [e^k v, e^k, e^-k v, e^-k]; suffix sums come from
total - prefix (total = prefix at the last grid point).  Fold e^{-+g} into
the tables, then for each query j with nearest grid point g_j and offset
d = q_j - g_j:
  num(q) = e^{-d}U + e^{d}V ~= (U+V) + d(V-U) = T0[g_j] + d*T1[g_j]
(first order in |d| <= DELTA/2), evaluated with a one-hot matmul whose
output lands TRANSPOSED ([128,16] per sample) so the division, h2 exchange
and output projection all stay in matmul-friendly layouts.  gamma/beta are
folded into the weights host-side.  Weights travel as fp8e4m3 (adds ~2e-3
rel err, tolerance is 2e-2).
"""

import os
import sys

import numpy as np

for _p in ("/opt/trn_rl_repo", "/root/.axon_site/_ro/trn_rl_repo"):
    if os.path.isdir(_p) and _p not in sys.path:
        sys.path.insert(0, _p)

import concourse.bass as bass
import concourse.tile as tile
from concourse import bacc, mybir
from concourse.bass_utils import run_bass_kernel_spmd

F32 = mybir.dt.float32
F16 = mybir.dt.float16
F8 = mybir.dt.float8e4
ALU = mybir.AluOpType
ACTF = mybir.ActivationFunctionType

B = 32
W = 2048
NCORES = 8
PCH = W // 128     # 16 feature chunks
FSL = W // NCORES  # 256
QKVW = 3 * FSL     # 768
SPC = B // NCORES  # 4 samples per core

G = 128
LO, HI = -8.0, 8.0
DELTA = (HI - LO) / (G - 1)
HALF = DELTA / 2.0
EPS = 1e-6

MODE = os.environ.get("ATTN_MODE", "fast")
MASK1 = os.environ.get("MASK1", "1") == "1"   # single stride-0 mask op
GROUPS = [list(range(NCORES))]


def _ap(handle, offset, ap):
    return bass.AP(tensor=handle, offset=offset, ap=ap)


def build(mode=None, reps=1, skip_gb=True):
    mode = mode or MODE
    nc = bacc.Bacc("TRN2", target_bir_lowering=False, debug=False,
                   num_devices=NCORES)

    x_t = nc.dram_tensor("x", [B, W], F16, kind="ExternalInput")
    wqkv_t = nc.dram_tensor("wqkv", [W, QKVW], F8, kind="ExternalInput")
    bqkv_t = nc.dram_tensor("bqkv", [QKVW], F16, kind="ExternalInput")
    wp_t = nc.dram_tensor("wp", [W, FSL], F8, kind="ExternalInput")
    xbp_t = nc.dram_tensor("xbp", [B, FSL], F32, kind="ExternalInput")
    out_t = nc.dram_tensor("out", [B, FSL], F32, kind="ExternalOutput")

    qkv_loc = nc.dram_tensor("qkv_loc", [B, QKVW], F16)
    qkv_a2a = nc.dram_tensor("qkv_a2a", [B, QKVW], F16)
    h2_loc = nc.dram_tensor("h2_loc", [128, SPC * PCH], F16)
    h2_gat = nc.dram_tensor("h2_gat", [NCORES * 128, SPC * PCH], F16,
                            addr_space="Shared")

    gridv = np.linspace(LO, HI, G, dtype=np.float64)
    c_eye8 = nc.inline_tensor(np.eye(8, dtype=np.float16), "c_eye8")
    c_gcoln = nc.inline_tensor((-gridv).astype(np.float32).reshape(G, 1),
                               "c_gcoln")
    c_gf16 = nc.inline_tensor(gridv.astype(np.float16).reshape(G, 1),
                              "c_gf16")
    c_egm = nc.inline_tensor(np.exp(-gridv).astype(np.float32).reshape(G, 1),
                             "c_egm")
    c_egp = nc.inline_tensor(np.exp(gridv).astype(np.float32).reshape(G, 1),
                             "c_egp")
    e127 = np.zeros((128, G), np.float32)
    e127[G - 1, :] = 1.0
    c_e127 = nc.inline_tensor(e127, "c_e127")
    c_gbig = nc.inline_tensor(
        np.repeat(gridv.astype(np.float16), PCH).reshape(1, G * PCH),
        "c_gbig")
    c_grow16 = nc.inline_tensor(gridv.astype(np.float16).reshape(1, G),
                                "c_grow16")

    aps = dict(
        x=x_t.ap(), wqkv=wqkv_t.ap(), bqkv=bqkv_t, wp=wp_t.ap(),
        xbp=xbp_t.ap(), out=out_t.ap(),
        qkv_loc=qkv_loc, qkv_a2a=qkv_a2a, h2_loc=h2_loc, h2_gat=h2_gat,
        eye8=c_eye8.ap(), gcoln=c_gcoln.ap(),
        gf16=c_gf16.ap(), egm=c_egm.ap(), egp=c_egp.ap(),
        e127=c_e127.ap(), gbig=c_gbig, grow16=c_grow16.ap(),
    )

    with tile.TileContext(nc) as tc:
        with tc.tile_pool(name="const", bufs=1) as cpool:
            cst = {}
            cst["eye8"] = cpool.tile([8, 8], F16, name="c_eye8s")
            nc.sync.dma_start(cst["eye8"][:], aps["eye8"])
            cst["gcoln"] = cpool.tile([G, 1], F32, name="c_gcolns")
            nc.sync.dma_start(cst["gcoln"][:], aps["gcoln"])
            cst["gf16"] = cpool.tile([G, 1], F16, name="c_gf16s")
            nc.sync.dma_start(cst["gf16"][:], aps["gf16"])
            cst["egm"] = cpool.tile([G, 1], F32, name="c_egms")
            nc.sync.dma_start(cst["egm"][:], aps["egm"])
            cst["egp"] = cpool.tile([G, 1], F32, name="c_egps")
            nc.sync.dma_start(cst["egp"][:], aps["egp"])
            cst["e127"] = cpool.tile([128, G], F32, name="c_e127s")
            nc.sync.dma_start(cst["e127"][:], aps["e127"])
            if MASK1:
                gbig_flat = cpool.tile([128, G * PCH], F16)
                nc.gpsimd.dma_start(
                    gbig_flat[:],
                    aps["gbig"].ap().partition_broadcast(128))
                a = gbig_flat[:]
                cst["gbigT"] = bass.AP(
                    tensor=a.tensor, offset=a.offset,
                    ap=[a.ap[0], [PCH, G], [1, PCH]])
            else:
                cst["gbc"] = cpool.tile([128, G], F16, name="c_gbcs")
                nc.gpsimd.dma_start(
                    cst["gbc"][:], aps["grow16"].partition_broadcast(128))
            for _rep in range(reps):
                _build_rep(tc, aps, cst)

    nc.compile()
    return nc


def _build_rep(tc, aps, cst):
    nc = tc.nc

    with tc.tile_pool(name="main", bufs=2) as mp, \
         tc.tile_pool(name="wpool", bufs=2) as wp_pool, \
         tc.tile_pool(name="qbp", bufs=2) as qb_pool, \
         tc.tile_pool(name="att", bufs=2) as ap_, \
         tc.tile_pool(name="ps_tr", bufs=2, space="PSUM") as ps_tr, \
         tc.tile_pool(name="ps_q", bufs=1, space="PSUM") as ps_q, \
         tc.tile_pool(name="ps_s", bufs=1, space="PSUM") as ps_s, \
         tc.tile_pool(name="ps_pj", bufs=2, space="PSUM") as ps_pj:

        # ---------------- layernorm ----------------
        sbx = mp.tile([B, W], F16, tag="sbx")
        nc.sync.dma_start(sbx[:], aps["x"])
        xg = sbx[:].rearrange("b (s f) -> b s f", s=4)
        stats = mp.tile([B, 4, 6], F32, tag="stats")
        for sg in range(4):
            nc.vector.bn_stats(stats[:, sg, :], xg[:, sg, :])
        mv = mp.tile([B, 2], F32, tag="mv")
        nc.vector.bn_aggr(mv[:], stats[:])
        eps_t = mp.tile([B, 1], F32, tag="eps")
        nc.vector.memset(eps_t[:], EPS)
        lnv = mp.tile([B, 1], F32, tag="lnv")
        nc.scalar.activation(lnv[:], mv[:, 1:2], ACTF.Ln, bias=eps_t[:])
        rstd = mp.tile([B, 1], F32, tag="rstd")
        nc.scalar.activation(rstd[:], lnv[:], ACTF.Exp, scale=-0.5)
        h = mp.tile([B, W], F16, tag="h")
        nc.vector.tensor_scalar(h[:], sbx[:], mv[:, 0:1], rstd[:],
                                op0=ALU.subtract, op1=ALU.mult)

        # -------- h transpose via DMA xbar: hT[p, ci, b] = h[b, ci*128+p] --
        hT = mp.tile([128, PCH, B], F16, tag="hT")
        nc.sync.dma_start_transpose(hT[:], h[:])

        # ---------------- qkv matmul ----------------
        wq_t = wp_pool.tile([128, PCH, QKVW], F8, tag="wq")
        nc.sync.dma_start(
            wq_t[:], _ap(aps["wqkv"].tensor, 0,
                         [[QKVW, 128], [128 * QKVW, PCH], [1, QKVW]]))
        pq = ps_q.tile([B, QKVW], F32, tag="pq")
        for ci in range(PCH):
            nc.tensor.matmul(pq[:, 0:512], hT[:, ci, :], wq_t[:, ci, 0:512],
                             start=(ci == 0), stop=(ci == PCH - 1))
            nc.tensor.matmul(pq[:, 512:QKVW], hT[:, ci, :],
                             wq_t[:, ci, 512:QKVW],
                             start=(ci == 0), stop=(ci == PCH - 1))
        # prefetch projection weights + residual early (overlaps collectives)
        wpb = wp_pool.tile([128, PCH, FSL], F8, tag="wpb")
        nc.sync.dma_start(
            wpb[:], _ap(aps["wp"].tensor, 0,
                        [[FSL, 128], [128 * FSL, PCH], [1, FSL]]))
        xbp_sb = mp.tile([B, FSL], F32, tag="xbp")
        nc.gpsimd.dma_start(xbp_sb[:], aps["xbp"])

        bqb = mp.tile([B, QKVW], F16, tag="bqb")
        nc.gpsimd.dma_start(bqb[:], aps["bqkv"].ap().partition_broadcast(B))
        sbq = mp.tile([B, QKVW], F16, tag="sbq")
        nc.vector.tensor_tensor(sbq[:], pq[:], bqb[:], op=ALU.add)
        nc.sync.dma_start(aps["qkv_loc"].ap(), sbq[:])

        # ---------------- AllToAll ----------------
        nc.gpsimd.collective_compute(
            "AllToAll", ALU.bypass, replica_groups=GROUPS,
            ins=[aps["qkv_loc"].ap()], outs=[aps["qkv_a2a"].ap()])

        a2a = aps["qkv_a2a"]

        # q rows broadcast: qb [128, SPC, W] f16 (one DMA per sample)
        qb = qb_pool.tile([128, SPC, W], F16, tag="qb")
        qb_eng = [nc.scalar, nc.sync, nc.scalar, nc.sync]
        for s in range(SPC):
            qb_eng[s].dma_start(
                qb[:, s, :],
                _ap(a2a, s * QKVW, [[0, 128], [4 * QKVW, 8], [1, FSL]]))
        # q/k/v rows natural: rows [8, SPC, 3, 256]
        rows = ap_.tile([8, SPC, 3, FSL], F16, tag="rows")
        for s in range(SPC):
            nc.gpsimd.dma_start(
                rows[:, s, :, :],
                _ap(a2a, s * QKVW, [[4 * QKVW, 8], [FSL, 3], [1, FSL]]))

        h2a = mp.tile([128, SPC, PCH], F16, tag="h2a")

        for s in range(SPC):
            # --- transpose q/k/v to [128, 16] ---
            tqkv = ap_.tile([128, 3, PCH], F16, tag="tqkv")
            for w in range(3):
                for hh in range(2):
                    ptq = ps_tr.tile([128, 8], F16, tag="ptq")
                    nc.tensor.transpose(
                        ptq[:], rows[:, s, w, hh * 128:(hh + 1) * 128],
                        cst["eye8"][:])
                    base = tqkv[:, w, :]
                    dst = bass.AP(tensor=base.tensor,
                                  offset=base.offset + hh,
                                  ap=[base.ap[0], [2, 8]])
                    if (w + hh) % 2 == 0:
                        nc.scalar.copy(dst, ptq[:])
                    else:
                        nc.vector.tensor_copy(dst, ptq[:])
            qT = tqkv[:, 0, :]
            kc = ap_.tile([128, PCH], F16, tag="kc")
            nc.vector.tensor_scalar(kc[:], tqkv[:, 1, :], LO, HI,
                                    op0=ALU.max, op1=ALU.min)

            # --- u columns [e^k v, e^k, e^-k v, e^-k] ---
            u = ap_.tile([128, PCH, 4], F16, tag="u")
            nc.scalar.activation(u[:, :, 1], kc[:], ACTF.Exp)
            nc.scalar.activation(u[:, :, 3], kc[:], ACTF.Exp, scale=-1.0)
            nc.vector.tensor_tensor(u[:, :, 0], u[:, :, 1], tqkv[:, 2, :],
                                    op=ALU.mult)
            nc.vector.tensor_tensor(u[:, :, 2], u[:, :, 3], tqkv[:, 2, :],
                                    op=ALU.mult)

            # --- mask + prefix tables S[g, r] ---
            if MASK1:
                mk = ap_.tile([128, G, PCH], F16, tag="mk")
                ka = kc[:]
                kb0 = bass.AP(tensor=ka.tensor, offset=ka.offset,
                              ap=[ka.ap[0], [0, G], [1, PCH]])
                nc.vector.tensor_tensor(mk[:], kb0, cst["gbigT"],
                                        ALU.is_le)

                def mk_lhsT(m):
                    a = mk[:]
                    return bass.AP(tensor=a.tensor, offset=a.offset + m,
                                   ap=[a.ap[0], [PCH, G]])
            else:
                mk = ap_.tile([128, PCH, G], F16, tag="mk")
                for m in range(PCH):
                    nc.vector.tensor_scalar(mk[:, m, :], cst["gbc"][:],
                                            kc[:, m:m + 1], None,
                                            op0=ALU.is_ge)

                def mk_lhsT(m):
                    return mk[:, m, :]

            SS = ps_s.tile([G, 8], F32, tag="SS")
            S = SS[:, 0:4]
            TBp = SS[:, 4:6]
            for m in range(PCH):
                nc.tensor.matmul(S, mk_lhsT(m), u[:, m, :],
                                 start=(m == 0), stop=(m == PCH - 1))
            Ssb = ap_.tile([G, 4], F32, tag="Ssb")
            nc.scalar.copy(Ssb[:], S)

            # broadcast totals (= prefix at the top grid row) to all rows
            nc.tensor.matmul(TBp, cst["e127"][:], Ssb[:, 2:4],
                             start=True, stop=True)
            V2 = ap_.tile([G, 2], F32, tag="V2")
            nc.vector.tensor_tensor(V2[:], TBp, Ssb[:, 2:4],
                                    op=ALU.subtract)
            U = ap_.tile([G, 2], F32, tag="U")
            nc.gpsimd.tensor_scalar(U[:], Ssb[:, 0:2], cst["egm"][:], None,
                                    op0=ALU.mult)
            V = ap_.tile([G, 2], F32, tag="V")
            nc.gpsimd.tensor_scalar(V[:], V2[:], cst["egp"][:], None,
                                    op0=ALU.mult)
            tabs = ap_.tile([G, 5], F16, tag="tabs")
            nc.vector.tensor_tensor(tabs[:, 0:2], U[:], V[:], op=ALU.add)
            nc.vector.tensor_tensor(tabs[:, 2:4], V[:], U[:],
                                    op=ALU.subtract)
            nc.gpsimd.tensor_copy(tabs[:, 4:5], cst["gf16"][:])

            # --- one-hot E ---
            t1 = qb_pool.tile([128, W], F16, tag="t1")
            nc.scalar.activation(t1[:], qb[:, s, :], ACTF.Abs,
                                 bias=cst["gcoln"][:])
            E = qb_pool.tile([128, W], F16, tag="E")
            nc.vector.tensor_scalar(E[:], t1[:], HALF, None, op0=ALU.is_le)

            # --- gather (transposed): PJ [128, 16, 5] ---
            PJ = ps_pj.tile([128, PCH, 5], F32, tag="PJ")
            for mj in range(PCH):
                nc.tensor.matmul(PJ[:, mj, :],
                                 E[:, mj * 128:(mj + 1) * 128], tabs[:],
                                 start=True, stop=True)

            # --- combine: h2 = (T0 + d T1) / (T2 + d T3) ---
            dn = ap_.tile([128, PCH], F32, tag="dn")
            nc.vector.tensor_tensor(dn[:], qT, PJ[:, :, 4], op=ALU.subtract)
            cn = ap_.tile([128, PCH], F32, tag="cn")
            nc.vector.tensor_tensor(cn[:], dn[:], PJ[:, :, 2], op=ALU.mult)
            num = ap_.tile([128, PCH], F32, tag="num")
            nc.vector.tensor_tensor(num[:], cn[:], PJ[:, :, 0], op=ALU.add)
            cd = ap_.tile([128, PCH], F32, tag="cd")
            nc.vector.tensor_tensor(cd[:], dn[:], PJ[:, :, 3], op=ALU.mult)
            den = ap_.tile([128, PCH], F32, tag="den")
            nc.vector.tensor_tensor(den[:], cd[:], PJ[:, :, 1], op=ALU.add)
            rden = ap_.tile([128, PCH], F32, tag="rden")
            nc.vector.reciprocal(rden[:], den[:])
            nc.vector.tensor_tensor(h2a[:, s, :], num[:], rden[:],
                                    op=ALU.mult)

        nc.gpsimd.dma_start(aps["h2_loc"].ap(), h2a[:])

        # ---------------- AllGather (transposed h2) ----------------
        nc.gpsimd.collective_compute(
            "AllGather", ALU.bypass, replica_groups=GROUPS,
            ins=[aps["h2_loc"].ap()], outs=[aps["h2_gat"].ap()])

        # h2g [128, 8, 64]: partition p, core c, (s*16+m)
        SW = SPC * PCH
        h2g = mp.tile([128, NCORES, SW], F16, tag="h2g")
        nc.sync.dma_start(
            h2g[:], _ap(aps["h2_gat"], 0,
                        [[SW, 128], [128 * SW, NCORES], [1, SW]]))

        # ---------------- output projection ----------------
        pout = ps_q.tile([B, FSL], F32, tag="pout")
        for ci in range(PCH):
            a = h2g[:]
            lhsT = bass.AP(tensor=a.tensor, offset=a.offset + ci,
                           ap=[a.ap[0], [SW, NCORES], [PCH, SPC]])
            nc.tensor.matmul(pout[:], lhsT, wpb[:, ci, :],
                             start=(ci == 0), stop=(ci == PCH - 1))
        outsb = mp.tile([B, FSL], F32, tag="outsb")
        nc.vector.tensor_tensor(outsb[:], pout[:], xbp_sb[:], op=ALU.add)
        nc.sync.dma_start(aps["out"], outsb[:])


_BUILT = {}


def _get_nc(mode, skip_gb=True):
    key = (mode, skip_gb)
    if key not in _BUILT:
        _BUILT[key] = build(mode, skip_gb=skip_gb)
    return _BUILT[key]


def make_in_maps(inputs):
    x = np.ascontiguousarray(np.asarray(inputs["x"], np.float32))
    gamma = np.asarray(inputs["gamma"], np.float32)
    beta = np.asarray(inputs["beta"], np.float32)
    Wq = np.asarray(inputs["Wq"], np.float32)
    Wk = np.asarray(inputs["Wk"], np.float32)
    Wv = np.asarray(inputs["Wv"], np.float32)
    Wp = np.asarray(inputs["Wp"], np.float32)
    bq = np.asarray(inputs["bq"], np.float32)
    bk = np.asarray(inputs["bk"], np.float32)
    bv = np.asarray(inputs["bv"], np.float32)
    bp = np.asarray(inputs["bp"], np.float32)

    # fold gamma/beta into the qkv weights: (h*gamma + beta) @ W + b
    #   = h @ (gamma[:,None]*W) + (beta @ W + b)
    g = gamma[:, None]
    Wqf, Wkf, Wvf = g * Wq, g * Wk, g * Wv
    bqf = beta @ Wq + bq
    bkf = beta @ Wk + bk
    bvf = beta @ Wv + bv

    f8 = mybir.dt.np(F8)
    in_maps = []
    for c in range(NCORES):
        cs = slice(c * FSL, (c + 1) * FSL)
        wqkv = np.concatenate([Wqf[:, cs], Wkf[:, cs], Wvf[:, cs]], axis=1)
        bqkv = np.concatenate([bqf[cs], bkf[cs], bvf[cs]])
        in_maps.append({
            "x": x.astype(np.float16),
            "wqkv": np.ascontiguousarray(wqkv).astype(f8),
            "bqkv": np.ascontiguousarray(bqkv).astype(np.float16),
            "wp": np.ascontiguousarray(Wp[:, cs]).astype(f8),
            "xbp": np.ascontiguousarray(x[:, cs] + bp[None, cs]),
        })
    return in_maps


def kernel(**inputs):
    nc = _get_nc(MODE)
    in_maps = make_in_maps(inputs)
    res = run_bass_kernel_spmd(nc, in_maps, core_ids=list(range(NCORES)))
    out = np.concatenate([res.results[c]["out"] for c in range(NCORES)],
                         axis=1)
    return np.ascontiguousarray(out.astype(np.float32))
